# revision 7
# baseline (speedup 1.0000x reference)
"""DCT heat-blur kernel for Trainium2 (8 NeuronCores, Bass/Tile).

Reference computes, per image X (one (batch, channel) slice):
    C = D X D^T;  C *= E2;  out = D C D^T        (note: D again, per the
einsum index order in the reference -- NOT D^T)  with E2 = e e^T rank-1,
e_h = exp(-(pi h/N)^2 t_b).  So out = M X M^T with M = D diag(e) D.

e decays fast for large sigma: only the first k ~ 224/sigma frequencies
survive at the accuracy gate.  Truncating, M ~= Dc_k E_k D_k with
D_k = D[:k,:], Dc_k = D[:,:k], E_k = diag(e[:k]), giving a 4-stage chain
per image that only touches k-sized intermediates:

    f1: S   = X^T D_k^T                [256, k]   (lhsT = X chunks)
    f2: C'  = D_k S = D_k X^T D_k^T    [k, k]     (lhsT = const D^T slices)
    c2: Cs  = E C'           (scale rows by e during PSUM->SBUF copy)
    f3: T3  = Cs^T (Dc_k)^T = C E Dc_k^T  [k,256] (rhs = const D^T rows)
    c3: T3s = E T3           (scale rows by e during PSUM->SBUF copy)
    f4: out = Dc_k T3s                 [256, 256] (lhsT = const D^T slices)

All constant operands are slices of ONE [128, 2, 256] D^T tile.  The
per-batch spectral decay e enters only via the two scaled copies (per-
partition scalar multiply), so nothing batch-specific is ever matmul'd.

Batches with sigma too small for k<=128 (kx = 224/sigma > 128) fall back
to the dense 2-GEMM path out = W^T X W, W = M^T host-built per batch.

Buckets are balanced across the 8 cores by promoting batches to larger-k
buckets until every bucket count is divisible by 8 (more frequencies is
always accuracy-safe), so all cores compile the identical program (SPMD).

I/O is fp16 both directions (host casts back to fp32); DMA drops from
21MB to ~13MB per core, which is the roofline here (~358 GB/s/core).
"""

import os
import numpy as np

BATCH = 128
CHANNELS = 3
N = 256
N_CORES = 8
PB = BATCH // N_CORES          # batches per core
IMGS = PB * CHANNELS           # images per core (48)
GRP = 4                        # images per DMA group
NG = IMGS // GRP               # groups per core (12)

K_COEF = 224.0                 # k needed ~ K_COEF / sigma (validated)
K_LEVELS = (128, 96, 64, 32)   # low-rank buckets (dense = 256 sentinel)
DENSE = 256

LAST_EXEC_TIME_NS = None
_NC_CACHE = {}


def _install_ntff_hook():
    """Wire antenv.axon_hooks (missing in this image) so trace=True works."""
    import sys
    import types

    if "antenv.axon_hooks" in sys.modules:
        return
    try:
        import trn_agent_boot.trn_boot as tb

        hook = tb._ntff_profile_via_ctypes("/opt/axon/libaxon_pjrt.so")
    except Exception:
        hook = None
    m = types.ModuleType("antenv.axon_hooks")
    m.get_axon_ntff_profile_hook = lambda: hook
    m.set_axon_ntff_profile_hook = lambda h: None
    sys.modules["antenv.axon_hooks"] = m


def _dct_matrix():
    n = np.arange(N, dtype=np.float64)
    D = np.sqrt(2.0 / N) * np.cos(np.pi * (n[None, :] + 0.5) * n[:, None] / N)
    D[0] *= 1.0 / np.sqrt(2.0)
    return D


def _bucket_of_sigma(sigma):
    """Frequency count needed for this sigma, as a bucket level (DENSE=fallback)."""
    if not np.isfinite(sigma) or sigma <= 0.0:
        return DENSE
    kx = K_COEF / float(sigma)
    if kx > 128.0:
        return DENSE
    for k in reversed(K_LEVELS):  # smallest first
        if kx <= k:
            return k
    return DENSE


def _build_nc(sig):
    """sig = (n_dense, n128, n96, n64, n32) batches PER CORE."""
    import concourse.bacc as bacc
    import concourse.tile as tile
    import concourse.mybir as mybir

    f32 = mybir.dt.float32
    f16 = mybir.dt.float16

    nd, n128, n96, n64, n32 = sig
    # per-core image bucket list, in processing order (low-rank first)
    img_k = []
    for k, cnt in ((128, n128), (96, n96), (64, n64), (32, n32)):
        img_k += [k] * (cnt * CHANNELS)
    img_k += [DENSE] * (nd * CHANNELS)
    assert len(img_k) == IMGS
    nl = n128 + n96 + n64 + n32      # low-rank batches per core

    nc = bacc.Bacc("TRN2", target_bir_lowering=False, debug=False)
    x_d = nc.dram_tensor("x", [NG, 128, GRP * 2 * N], f16, kind="ExternalInput").ap()
    dt_d = nc.dram_tensor("dt", [128, 2 * N], f16, kind="ExternalInput").ap()
    if nl:
        e_d = nc.dram_tensor("e", [128, nl], f32, kind="ExternalInput").ap()
    if nd:
        w_d = nc.dram_tensor("w", [128, nd, 2, N], f16, kind="ExternalInput").ap()
    o_d = nc.dram_tensor("o", [NG, 128, GRP * 2 * N], f16, kind="ExternalOutput").ap()

    with tile.TileContext(nc) as tc:
        with (
            tc.tile_pool(name="const", bufs=1) as cpool,
            tc.tile_pool(name="xpool", bufs=NG + 1) as xpool,
            tc.tile_pool(name="spool", bufs=4) as spool,
            tc.tile_pool(name="cspool", bufs=4) as cspool,
            tc.tile_pool(name="t3spool", bufs=4) as t3spool,
            tc.tile_pool(name="t1pool", bufs=2) as t1pool,
            tc.tile_pool(name="opool", bufs=3) as opool,
            tc.tile_pool(name="ps1", bufs=2, space="PSUM") as ps1,
            tc.tile_pool(name="ps2", bufs=3, space="PSUM") as ps2,
            tc.tile_pool(name="pso", bufs=2, space="PSUM") as pso,
        ):
            # constants first, high priority
            with tc.high_priority():
                dt_sb = cpool.tile([128, 2, N], f16, name="dt")
                nc.sync.dma_start(dt_sb[:], dt_d.rearrange("p (a w) -> p a w", a=2))
            if nl:
                e_sb = cpool.tile([128, nl], f32, name="e")
                nc.sync.dma_start(e_sb[:], e_d)
            if nd:
                w_sb = cpool.tile([128, nd, 2, N], f16, name="w")
                nc.scalar.dma_start(w_sb[:], w_d)

            xt_tiles = {}

            def issue_load(g):
                xt = xpool.tile([128, GRP, 2, N], f16)
                nc.sync.dma_start(
                    xt[:], x_d[g].rearrange("p (i a w) -> p i a w", i=GRP, a=2)
                )
                xt_tiles[g] = xt

            for g in range(NG):
                issue_load(g)

            for g in range(NG):
                xt = xt_tiles.pop(g)
                ot = opool.tile([128, GRP, 2, N], f16)
                for ii in range(GRP):
                    img = g * GRP + ii
                    k = img_k[img]
                    if k == DENSE:
                        bi = (img - nl * CHANNELS) // CHANNELS  # dense batch idx
                        t1_ps = ps1.tile([128, 2, N], f32, tag="s1", name="t1ps")
                        for mb in range(2):
                            for a in range(2):
                                nc.tensor.matmul(
                                    t1_ps[:, mb, :],
                                    lhsT=xt[:, ii, a, mb * 128 : (mb + 1) * 128],
                                    rhs=w_sb[:, bi, a, :],
                                    start=(a == 0),
                                    stop=(a == 1),
                                )
                        t1_sb = t1pool.tile([128, 2, N], f16)
                        nc.vector.tensor_copy(out=t1_sb[:, 0], in_=t1_ps[:, 0])
                        nc.scalar.copy(t1_sb[:, 1], t1_ps[:, 1])
                        o_ps = pso.tile([128, 2, N], f32, tag="o", name="ops")
                        for mb in range(2):
                            for a in range(2):
                                nc.tensor.matmul(
                                    o_ps[:, mb, :],
                                    lhsT=t1_sb[:, a, mb * 128 : (mb + 1) * 128],
                                    rhs=w_sb[:, bi, a, :],
                                    start=(a == 0),
                                    stop=(a == 1),
                                )
                        nc.scalar.copy(ot[:, ii, 0], o_ps[:, 0])
                        nc.vector.tensor_copy(out=ot[:, ii, 1], in_=o_ps[:, 1])
                    else:
                        bi = img // CHANNELS               # low-rank batch idx
                        ev = e_sb[0:k, bi : bi + 1]
                        # f1: S = X^T D_k^T  [2mb x 128, k]
                        s1_ps = ps1.tile(
                            [128, 2, k], f32, tag="s1", padded_shape=[128, 2, N],
                            name="s1ps",
                        )
                        for mb in range(2):
                            for a in range(2):
                                nc.tensor.matmul(
                                    s1_ps[:, mb, :],
                                    lhsT=xt[:, ii, a, mb * 128 : (mb + 1) * 128],
                                    rhs=dt_sb[:, a, 0:k],
                                    start=(a == 0),
                                    stop=(a == 1),
                                )
                        par = img % 2
                        eng = (nc.vector, nc.scalar) if par else (nc.scalar, nc.vector)
                        s_sb = spool.tile([128, 2, k], f16)
                        if par:
                            nc.vector.tensor_copy(out=s_sb[:], in_=s1_ps[:])
                        else:
                            nc.scalar.copy(s_sb[:], s1_ps[:])
                        # f2: C' = D_k S  [k, k]
                        c_ps = ps2.tile(
                            [k, k], f32, tag="m", padded_shape=[128, 2 * N],
                            name="cps",
                        )
                        for a in range(2):
                            nc.tensor.matmul(
                                c_ps[:],
                                lhsT=dt_sb[:, a, 0:k],
                                rhs=s_sb[:, a, :],
                                start=(a == 0),
                                stop=(a == 1),
                            )
                        cs_sb = cspool.tile([k, k], f16)
                        if par:
                            nc.scalar.mul(cs_sb[:], c_ps[:], ev)
                        else:
                            nc.vector.tensor_scalar_mul(cs_sb[:], c_ps[:], ev)
                        # f3: T3 = Cs^T (D^T rows 0:k)  [k, 256]
                        t3_ps = ps2.tile(
                            [k, N], f32, tag="m", padded_shape=[128, 2 * N],
                            name="t3ps",
                        )
                        nc.tensor.matmul(
                            t3_ps[:],
                            lhsT=cs_sb[:],
                            rhs=dt_sb[0:k, 0, :],
                            start=True,
                            stop=True,
                        )
                        t3s_sb = t3spool.tile([k, N], f16)
                        if par:
                            nc.vector.tensor_scalar_mul(t3s_sb[:], t3_ps[:], ev)
                        else:
                            nc.scalar.mul(t3s_sb[:], t3_ps[:], ev)
                        # f4: out = Dc_k T3s  [2mb x 128, 256]
                        o_ps = pso.tile([128, 2, N], f32, tag="o", name="ops")
                        for mb in range(2):
                            nc.tensor.matmul(
                                o_ps[:, mb, :],
                                lhsT=dt_sb[0:k, 0, mb * 128 : (mb + 1) * 128],
                                rhs=t3s_sb[:],
                                start=True,
                                stop=True,
                            )
                        if par:
                            nc.scalar.copy(ot[:, ii, 0], o_ps[:, 0])
                            nc.vector.tensor_copy(out=ot[:, ii, 1], in_=o_ps[:, 1])
                        else:
                            nc.vector.tensor_copy(out=ot[:, ii, 0], in_=o_ps[:, 0])
                            nc.scalar.copy(ot[:, ii, 1], o_ps[:, 1])
                nc.sync.dma_start(
                    o_d[g].rearrange("p (i a w) -> p i a w", i=GRP, a=2), ot[:]
                )

    nc.compile()
    return nc


def _get_nc(sig):
    if sig not in _NC_CACHE:
        _NC_CACHE[sig] = _build_nc(sig)
    return _NC_CACHE[sig]


def kernel(x, blur_sigmas, fwd_steps):
    global LAST_EXEC_TIME_NS
    from concourse import bass_utils

    x = np.asarray(x)
    assert x.shape == (BATCH, CHANNELS, N, N), x.shape
    sigmas = np.asarray(blur_sigmas, dtype=np.float64)
    steps = np.asarray(fwd_steps).astype(np.int64)
    bsig = sigmas[steps]                       # per-batch sigma

    # --- bucket assignment + promotion so every count is divisible by 8 ---
    levels = [DENSE] + list(K_LEVELS)          # big -> small
    buckets = {L: [] for L in levels}
    for b in range(BATCH):
        buckets[_bucket_of_sigma(bsig[b])].append(b)
    for i, L in enumerate(levels[:-1]):        # promote upward from smaller k
        short = (-len(buckets[L])) % N_CORES
        lower = [buckets[M] for M in levels[i + 1 :] if buckets[M]]
        j = 0
        while j < short and lower:
            src = lower[0]
            buckets[L].append(src.pop(0))      # front = largest k-need first
            if not src:
                lower.pop(0)
            j += 1
    assert all(len(v) % N_CORES == 0 for v in buckets.values()), {
        k: len(v) for k, v in buckets.items()
    }

    sig = tuple(len(buckets[L]) // N_CORES for L in levels)  # (nd,n128,n96,n64,n32)
    nd, n128, n96, n64, n32 = sig
    nl = n128 + n96 + n64 + n32

    # per-core batch lists in processing order: k128, k96, k64, k32, dense
    core_batches = []
    for c in range(N_CORES):
        lst = []
        for L in (128, 96, 64, 32, DENSE):
            v = buckets[L]
            m = len(v) // N_CORES
            lst += v[c * m : (c + 1) * m]
        core_batches.append(lst)

    # --- host-side constants / per-batch data ---
    D = _dct_matrix()
    freqs = np.pi * np.arange(N, dtype=np.float64) / N
    dt_host = np.ascontiguousarray(
        D.T.reshape(2, 128, N).transpose(1, 0, 2).reshape(128, 2 * N)
    ).astype(np.float16)  # dt[p, a*N+j] = D^T[a*128+p, j] = D[j, a*128+p]

    t_all = (bsig**2) / 2.0
    uniq_steps, inv = np.unique(steps, return_inverse=True)
    e_uniq = np.exp(-(freqs[None, :] ** 2) * (sigmas[uniq_steps][:, None] ** 2) / 2.0)
    w_uniq = {}
    for i, s in enumerate(uniq_steps):
        if _bucket_of_sigma(sigmas[s]) == DENSE:
            M = D @ (e_uniq[i][:, None] * D)
            w_uniq[i] = np.ascontiguousarray(
                M.T.astype(np.float16).reshape(2, 128, N).transpose(1, 0, 2)
            )  # [128, 2, N]: [p, a, h] = W[a*128+p, h], W = M^T

    x16 = x.astype(np.float16)
    in_maps = []
    for c in range(N_CORES):
        bl = core_batches[c]
        # image order: batches in bl order, channels inner
        imgs = x16[bl]                          # [PB, C, N, N]
        xc = (
            imgs.reshape(PB * CHANNELS, 2, 128, N)
            .transpose(2, 0, 1, 3)
            .reshape(128, NG, GRP * 2 * N)
            .transpose(1, 0, 2)
        )
        m = {"x": np.ascontiguousarray(xc), "dt": dt_host}
        if nl:
            e_host = np.zeros((128, nl), dtype=np.float32)
            for j in range(nl):
                e_host[:, j] = np.exp(-(freqs[:128] ** 2) * t_all[bl[j]])
            m["e"] = e_host
        if nd:
            w_host = np.zeros((128, nd, 2, N), dtype=np.float16)
            for j in range(nd):
                w_host[:, j] = w_uniq[inv[bl[nl + j]]]
            m["w"] = w_host
        in_maps.append(m)

    nc = _get_nc(sig)
    trace = os.environ.get("BASS_DCT_TRACE", "0") == "1"
    kwargs = {}
    if trace:
        _install_ntff_hook()
        kwargs["trace"] = True
        tmpdir = os.environ.get("BASS_DCT_TRACE_DIR")
        if tmpdir:
            kwargs["tmpdir"] = tmpdir
    res = None
    for attempt in range(3):
        try:
            res = bass_utils.run_bass_kernel_spmd(
                nc, in_maps, core_ids=list(range(N_CORES)), **kwargs
            )
            break
        except Exception:
            # transient NRT_EXEC_UNIT_UNRECOVERABLE has been observed on the
            # first execution of a freshly loaded NEFF; a retry succeeds
            if attempt == 2:
                raise
            import time as _time

            _time.sleep(2.0)
            kwargs.pop("trace", None)
            kwargs.pop("tmpdir", None)
    LAST_EXEC_TIME_NS = res.exec_time_ns

    out = np.empty((BATCH, CHANNELS, N, N), dtype=np.float32)
    for c in range(N_CORES):
        oc = res.results[c]["o"]               # [NG, 128, GRP*2*N] fp16
        oi = (
            oc.transpose(1, 0, 2)
            .reshape(128, PB * CHANNELS, 2, N)
            .transpose(1, 2, 0, 3)
            .reshape(PB, CHANNELS, N, N)
        )
        out[core_batches[c]] = oi.astype(np.float32)
    return out


# revision 8
# speedup vs baseline: 1.5817x; 1.5817x over previous
"""DCT heat-blur kernel for Trainium2 (8 NeuronCores, Bass/Tile).

Math: reference computes, per image X (one (batch, channel) slice):
    coefs = D X D^T;  coefs *= E;  out = D coefs D^T
with E[h,w] = exp(-(f_h^2 + f_w^2) t_b) = e e^T rank-1.  The elementwise
decay therefore factors through the transforms:
    out = M X M^T,  M = D diag(e) D;  device computes W^T X W, W = M^T.
W_b is a tiny per-batch 256x256 matrix built on host.  The device does
2 GEMMs per image instead of 4 + an elementwise pass.

Device layout per 256x256 image: row-blocks a=0,1 of 128 rows each.
apply(A, R)[m,h] = sum_k A[k,m] R[k,h] = (A^T R)[m,h] via
matmul(out[mb], lhsT=A[:, a, mb*128:(mb+1)*128], rhs=R[:, a, :]) summed
over a.  out = apply(apply(X, W), W).

Matmuls run in fp16 (full PE rate); I/O is fp16 BOTH directions -- the
host casts the fp16 result back to fp32.  This halves output DMA vs the
fp32 original (per-core DMA 21MB -> 14.7MB), taking DMA off the
critical path (PE weight-load throughput is the floor).

x / out are pre/post-permuted on host into the exact SBUF layout so all
big DMAs are fully contiguous.

Sharding: pure data parallel over batch, 16 batches (48 images) per core.
"""

import os
import numpy as np

BATCH = 128
CHANNELS = 3
N = 256
N_CORES = 8
PB = BATCH // N_CORES          # batches per core
IMGS = PB * CHANNELS           # images per core
GRP = 4                        # images per DMA group
NG = IMGS // GRP               # groups per core

LAST_EXEC_TIME_NS = None
_NC_CACHE = {}


def _install_ntff_hook():
    """Wire antenv.axon_hooks (missing in this image) so trace=True works."""
    import sys
    import types

    if "antenv.axon_hooks" in sys.modules:
        return
    try:
        import trn_agent_boot.trn_boot as tb

        hook = tb._ntff_profile_via_ctypes("/opt/axon/libaxon_pjrt.so")
    except Exception:
        hook = None
    m = types.ModuleType("antenv.axon_hooks")
    m.get_axon_ntff_profile_hook = lambda: hook
    m.set_axon_ntff_profile_hook = lambda h: None
    sys.modules["antenv.axon_hooks"] = m


def _build_nc():
    import concourse.bacc as bacc
    import concourse.tile as tile
    import concourse.mybir as mybir

    f32 = mybir.dt.float32
    f16 = mybir.dt.float16

    nc = bacc.Bacc("TRN2", target_bir_lowering=False, debug=False)
    # x/o are host-permuted: [group][partition][img_in_grp, rowblk, col]
    x_d = nc.dram_tensor("x", [NG, 128, GRP * 2 * N], f16, kind="ExternalInput").ap()
    # w: host-built per-batch W matrices, [partition][batch, rowblk, col]
    w_d = nc.dram_tensor("w", [128, PB, 2, N], f16, kind="ExternalInput").ap()
    o_d = nc.dram_tensor("o", [NG, 128, GRP * 2 * N], f16, kind="ExternalOutput").ap()

    PREFETCH = NG

    with tile.TileContext(nc) as tc:
        with (
            tc.tile_pool(name="const", bufs=1) as cpool,
            tc.tile_pool(name="xpool", bufs=PREFETCH + 1) as xpool,
            tc.tile_pool(name="tpool", bufs=6) as tpool,
            tc.tile_pool(name="opool", bufs=6) as opool,
            tc.tile_pool(name="ps1", bufs=4, space="PSUM") as ps1,
            tc.tile_pool(name="ps2", bufs=4, space="PSUM") as ps2,
        ):
            # fp16 loads are tiny and fully prefetched on sync; late store
            # issues go via the sync engine/ring (idle once loads finish)
            # so they never wait behind ACT's copy work
            def st_ring(g):
                return nc.scalar if g < NG // 2 else nc.sync

            xt_tiles = {}

            def issue_load(g):
                xt = xpool.tile([128, GRP, 2, N], f16)
                nc.sync.dma_start(
                    xt[:], x_d[g].rearrange("p (i a w) -> p i a w", i=GRP, a=2)
                )
                xt_tiles[g] = xt

            # host-built W in four INDEPENDENT quarter tiles so early
            # groups depend only on their own chunk's DMA (a single tile
            # would make the first matmul wait for all four chunk writes)
            w_q = []
            with tc.high_priority():
                wq0 = cpool.tile([128, 4, 2, N], f16, name="wq0")
                nc.sync.dma_start(wq0[:], w_d[:, 0:4])
                w_q.append(wq0)
            for q in range(1, 4):
                wq = cpool.tile([128, 4, 2, N], f16, name=f"wq{q}")
                nc.scalar.dma_start(wq[:], w_d[:, 4 * q : 4 * (q + 1)])
                w_q.append(wq)
            for g in range(PREFETCH):
                issue_load(g)

            for g in range(NG):
                if g + PREFETCH < NG:
                    issue_load(g + PREFETCH)
                xt = xt_tiles.pop(g)
                ot = opool.tile([128, GRP, 2, N], f16)
                for ii in range(GRP):
                    img = g * GRP + ii
                    b = img // CHANNELS
                    t1_ps = ps1.tile([128, 2, N], f32)
                    for mb in range(2):
                        for a in range(2):
                            nc.tensor.matmul(
                                t1_ps[:, mb, :],
                                lhsT=xt[:, ii, a, mb * 128 : (mb + 1) * 128],
                                rhs=w_q[b // 4][:, b % 4, a, :],
                                start=(a == 0),
                                stop=(a == 1),
                            )
                    t1_sb = tpool.tile([128, 2, N], f16)
                    if ii % 2 == 0:
                        nc.vector.tensor_copy(out=t1_sb[:], in_=t1_ps[:])
                    else:
                        nc.scalar.copy(t1_sb[:], t1_ps[:])
                    t2_ps = ps2.tile([128, 2, N], f32)
                    for mb in range(2):
                        for a in range(2):
                            nc.tensor.matmul(
                                t2_ps[:, mb, :],
                                lhsT=t1_sb[:, a, mb * 128 : (mb + 1) * 128],
                                rhs=w_q[b // 4][:, b % 4, a, :],
                                start=(a == 0),
                                stop=(a == 1),
                            )
                    if ii % 2 == 0:
                        nc.scalar.copy(ot[:, ii], t2_ps[:])
                    else:
                        nc.vector.tensor_copy(out=ot[:, ii], in_=t2_ps[:])
                    # late groups store per image from the idle sync ring so
                    # the final drain is one 256KB piece, not a 1MB group
                    if g >= NG // 2:
                        nc.sync.dma_start(
                            o_d[g].rearrange(
                                "p (i a w) -> p i a w", i=GRP, a=2
                            )[:, ii],
                            ot[:, ii],
                        )
                if g < NG // 2:
                    st_ring(g).dma_start(
                        o_d[g].rearrange("p (i a w) -> p i a w", i=GRP, a=2), ot[:]
                    )

    nc.compile()
    return nc


def _get_nc():
    key = "nc_v2"
    if key not in _NC_CACHE:
        _NC_CACHE[key] = _build_nc()
    return _NC_CACHE[key]


def _host_w(blur_sigmas, fwd_steps):
    """Per-batch W_b = (D diag(e_b) D)^T in device layout [128, B, 2, N]."""
    sig = np.asarray(blur_sigmas, dtype=np.float64)
    steps = np.asarray(fwd_steps).astype(np.int64)
    n = np.arange(N, dtype=np.float64)
    D = np.sqrt(2.0 / N) * np.cos(np.pi * (n[None, :] + 0.5) * n[:, None] / N)
    D[0] *= 1.0 / np.sqrt(2.0)
    freqs = np.pi * n / N
    uniq, inv = np.unique(steps, return_inverse=True)
    ms = np.empty((len(uniq), N, N), dtype=np.float16)
    for i, s in enumerate(uniq):
        t = sig[s] ** 2 / 2.0
        e = np.exp(-(freqs**2) * t)
        w = (D @ (e[:, None] * D)).T
        ms[i] = w.astype(np.float16)
    w_all = ms[inv]  # [B, N, N]
    # device layout [128, B, 2, N]: [p, b, a, h] = W_b[a*128+p, h]
    return np.ascontiguousarray(
        w_all.reshape(BATCH, 2, 128, N).transpose(2, 0, 1, 3)
    )


def kernel(x, blur_sigmas, fwd_steps):
    global LAST_EXEC_TIME_NS
    from concourse import bass_utils

    x = np.asarray(x)
    assert x.shape == (BATCH, CHANNELS, N, N), x.shape
    x = x.astype(np.float16)
    w_host = _host_w(blur_sigmas, fwd_steps)

    # device x layout: [core][NG, 128, GRP*2*N]
    # x[img, a*128+p, w] -> xc[g, p, (i, a, w)]
    xp = (
        x.reshape(N_CORES, NG, GRP, 2, 128, N)
        .transpose(0, 1, 4, 2, 3, 5)
        .reshape(N_CORES, NG, 128, GRP * 2 * N)
    )
    in_maps = []
    for i in range(N_CORES):
        in_maps.append(
            {
                "x": np.ascontiguousarray(xp[i]),
                "w": np.ascontiguousarray(w_host[:, i * PB : (i + 1) * PB]),
            }
        )

    nc = _get_nc()
    trace = os.environ.get("BASS_DCT_TRACE", "0") == "1"
    kwargs = {}
    if trace:
        _install_ntff_hook()
        kwargs["trace"] = True
        tmpdir = os.environ.get("BASS_DCT_TRACE_DIR")
        if tmpdir:
            kwargs["tmpdir"] = tmpdir
    res = None
    for attempt in range(3):
        try:
            res = bass_utils.run_bass_kernel_spmd(
                nc, in_maps, core_ids=list(range(N_CORES)), **kwargs
            )
            break
        except Exception:
            # transient NRT_EXEC_UNIT_UNRECOVERABLE has been observed on the
            # first execution of a freshly loaded NEFF; a retry succeeds
            if attempt == 2:
                raise
            import time as _time

            _time.sleep(2.0)
            kwargs.pop("trace", None)
            kwargs.pop("tmpdir", None)
    LAST_EXEC_TIME_NS = res.exec_time_ns

    # inverse permute: oc[g, p, (i, a, w)] -> out[img, a*128+p, w]
    oc = np.stack([res.results[i]["o"] for i in range(N_CORES)])
    out = (
        oc.reshape(N_CORES, NG, 128, GRP, 2, N)
        .transpose(0, 1, 3, 4, 2, 5)
        .reshape(BATCH, CHANNELS, N, N)
    )
    return np.ascontiguousarray(out.astype(np.float32))
